# revision 5
# baseline (speedup 1.0000x reference)
"""TSM-style 3-tap depthwise temporal conv on 8 Trainium2 NeuronCores.

out[n, t, c, h, w] = w[c,0]*x[n,t-1,c,h,w] + w[c,1]*x[n,t,c,h,w]
                   + w[c,2]*x[n,t+1,c,h,w]   (zero-padded at clip edges)

Sharding: pure data parallel over the nt (clip-batch) axis — each of the 8
cores gets whole clips (nt=64, n_segment=8 -> one 8-frame clip per core).
Weight (c,3) is replicated.

Platform notes (measured on this axon-tunneled TRN2, see mb.py):
- Engines and DMA queues serialize globally (no overlap anywhere; mixing
  engines/queues adds ~15-25us per switch). Total time ~= sum of per-op
  costs, so the kernel minimizes instruction count and per-op cost.
- DMA cost ~= 55ns/descriptor + bytes at ~230GB/s. The (f c x -> c f x)
  transposed load/store is descriptor-bound (128 part x 8 frames = 1024
  descriptors of 12544B per 12.8MB block); this layout is forced by the
  per-partition weight scalars, and splitting tiles only adds descriptors.
- DVE scalar_tensor_tensor (tt-class, ~3.8ns/elem) is ~3x the per-element
  cost of tensor_scalar (ts-class, ~1.2ns/elem); fp16 does NOT speed up
  either class (measured slower for ts-class), so compute stays fp32.
- Flattening the stt slices to 1D APs (rows are contiguous per partition)
  measured ~12% faster than the equivalent 2D APs (84us -> ~73us).

Per channel-block of 128 channels: one 12.8MB load, y = x*w1 (ts-class),
two flat-AP stt taps (y[t] += w0*x[t-1], y[t] += w2*x[t+1]), one store.
10 instructions per core per pass; bulk DMA stays on the single gpsimd
(SWDGE) queue (measured cheapest, and avoids queue switches); all compute
on DVE.
"""

import numpy as np

import concourse.bacc as bacc
import concourse.mybir as mybir
import concourse.tile as tile
from concourse.bass_utils import run_bass_kernel_spmd

N_CORES = 8
P = 128  # SBUF partitions

MULT = mybir.AluOpType.mult
ADD = mybir.AluOpType.add

_cache = {}


def _emit_conv(nc, tc, pools, src, dst, wt_by_blk, F, C, HW, n_seg, uid):
    """Emit one full conv pass src -> dst (both DRAM (F, C, HW) handles)."""
    wp, xp, yp = pools
    nblk = C // P
    n_clips = max(F // n_seg, 1)
    S = min(n_seg, F)

    for b in range(nblk):
        cs = slice(b * P, (b + 1) * P)
        wt = wt_by_blk[b]
        w0, w1, w2 = wt[:, 0:1], wt[:, 1:2], wt[:, 2:3]

        xt = xp.tile([P, F, HW], mybir.dt.float32, tag="x", name=f"x{uid}_{b}")
        nc.gpsimd.dma_start(
            out=xt[:], in_=src[:, cs, :].rearrange("f c x -> c f x"))

        y = yp.tile([P, F, HW], mybir.dt.float32, tag="y", name=f"y{uid}_{b}")
        # flat 1D views (rows contiguous within a partition) — measured
        # faster than the equivalent (f, x) 2D APs
        xf = xt[:].rearrange("c f x -> c (f x)")
        yf = y[:].rearrange("c f x -> c (f x)")
        nc.vector.tensor_scalar_mul(yf[:], xf[:], w1)
        for c in range(n_clips):
            lo = c * S * HW
            n1 = (S - 1) * HW
            nc.vector.scalar_tensor_tensor(
                yf[:, lo + HW : lo + HW + n1], xf[:, lo : lo + n1], w0,
                yf[:, lo + HW : lo + HW + n1], MULT, ADD)
            nc.vector.scalar_tensor_tensor(
                yf[:, lo : lo + n1], xf[:, lo + HW : lo + HW + n1], w2,
                yf[:, lo : lo + n1], MULT, ADD)

        nc.gpsimd.dma_start(
            out=dst[:, cs, :].rearrange("f c x -> c f x"), in_=y[:])


def _build(F, C, HW, n_seg, repeat=1, x_bufs=1, y_bufs=1):
    """One-core program: x (F, C, HW) -> out (F, C, HW).

    repeat > 1 chains the conv through internal DRAM ping-pong buffers —
    identical HBM traffic per pass; used by the timing harness.
    """
    nc = bacc.Bacc(
        "TRN2",
        target_bir_lowering=False,
        debug=False,
        num_devices=N_CORES,
    )
    x = nc.dram_tensor("x", (F, C, HW), mybir.dt.float32, kind="ExternalInput")
    w = nc.dram_tensor("weight", (C, 3), mybir.dt.float32, kind="ExternalInput")
    out = nc.dram_tensor("out", (F, C, HW), mybir.dt.float32, kind="ExternalOutput")
    scratch = [
        nc.dram_tensor(f"scratch{i}", (F, C, HW), mybir.dt.float32, kind="Internal")
        for i in range(2 if repeat > 1 else 0)
    ]

    nblk = C // P
    with tile.TileContext(nc) as tc:
        with (
            tc.tile_pool(name="wp", bufs=1) as wp,
            tc.tile_pool(name="xp", bufs=x_bufs) as xp,
            tc.tile_pool(name="yp", bufs=y_bufs) as yp,
        ):
            # all channel-blocks' weights in one DMA: partition p holds
            # channels p, p+128, ... as (nblk, 3) in the free dim
            # weight load on the same gpsimd queue as all bulk DMA — avoids
            # one queue transition (~20us switch penalty on this platform)
            wtile = wp.tile([P, nblk, 3], mybir.dt.float32, tag="w", name="wtile")
            nc.gpsimd.dma_start(
                out=wtile[:], in_=w.ap().rearrange("(b c) k -> c b k", c=P)
            )
            wt_by_blk = [wtile[:, b, :] for b in range(nblk)]

            pools = (wp, xp, yp)
            for k in range(repeat):
                src = x if k == 0 else scratch[k % 2]
                dst = out if k == repeat - 1 else scratch[(k + 1) % 2]
                _emit_conv(nc, tc, pools, src, dst, wt_by_blk, F, C, HW, n_seg, k)
    nc.compile()
    return nc


def _get_program(F, C, HW, n_seg, repeat=1):
    key = (F, C, HW, n_seg, repeat)
    if key not in _cache:
        _cache[key] = _build(F, C, HW, n_seg, repeat=repeat)
    return _cache[key]


def kernel(x, weight, n_segment, **_kw):
    x = np.asarray(x)
    weight = np.ascontiguousarray(np.asarray(weight, dtype=np.float32))
    n_seg = int(np.asarray(n_segment))
    nt, C, H, W = x.shape
    HW = H * W
    assert nt % N_CORES == 0
    F = nt // N_CORES
    # each core must hold whole clips
    assert F % n_seg == 0 or n_seg % F == 0, (F, n_seg)

    nc = _get_program(F, C, HW, n_seg)

    xs = np.ascontiguousarray(x, dtype=np.float32).reshape(nt, C, HW)
    in_maps = [
        {"x": xs[i * F : (i + 1) * F], "weight": weight} for i in range(N_CORES)
    ]
    res = run_bass_kernel_spmd(nc, in_maps, list(range(N_CORES)))
    out = np.concatenate([res.results[i]["out"] for i in range(N_CORES)], axis=0)
    return out.reshape(nt, C, H, W).astype(x.dtype, copy=False)
